# revision 1
# baseline (speedup 1.0000x reference)
"""Causal self-attention kernel v3 for 8 Trainium2 NeuronCores (Bass/Tile).

B=4, T=2048, C=1024, 16 heads. 8 cores = 4 batches x 2 head-groups (8 heads
each); host sums the two projection partials per batch.

Changes vs baseline:
- all-bf16 matmul operands (hides LDWEIGHTS, halves DMA/SBUF traffic)
- causal trimming of scores / exp / att*v at 128-block granularity,
  no masked-region memsets
- att*v emission delayed one key-block behind scores so the PE never
  waits on the scalar-engine exp
- softmax normalization: reciprocal on a [128,8] scatter of the
  denominators (was [128,512]), gpsimd partition-broadcast, in-place
  scaling of y
- projection PSUM shares the scores pool; y/den PSUM double-buffered
"""


import numpy as np
import concourse.bass as bass
import concourse.tile as tile
from concourse import mybir, bacc

F32 = mybir.dt.float32
BF16 = mybir.dt.bfloat16


def build(T=2048, HL=8, C=1024):
    D = 64
    HP = HL // 2               # head pairs per core
    NCK = C // 128             # contraction chunks for qkv
    NI = T // 512              # 512-wide query blocks
    NTK = T // 128             # 128-wide key blocks

    nc = bacc.Bacc("TRN2", debug=False, num_devices=8)

    xt = nc.dram_tensor("xt", [NCK, 128, T], BF16, kind="ExternalInput")
    wq = nc.dram_tensor("wq", [NCK, 128, HL * D], BF16, kind="ExternalInput")
    wk = nc.dram_tensor("wk", [NCK, 128, HL * D], BF16, kind="ExternalInput")
    wv = nc.dram_tensor("wv", [NCK, 128, HL * D], BF16, kind="ExternalInput")
    wp = nc.dram_tensor("wp", [HP, 128, C], BF16, kind="ExternalInput")
    tri = nc.dram_tensor("tri", [128, 256], BF16, kind="ExternalInput")
    ident = nc.dram_tensor("ident", [128, 128], BF16, kind="ExternalInput")
    out = nc.dram_tensor("out", [T, C], F32, kind="ExternalOutput")

    with tile.TileContext(nc) as tc:
        with (
            tc.tile_pool(name="persist", bufs=1) as pers,
            tc.tile_pool(name="qkv", bufs=1) as qkvp,
        ):
            id_sb = pers.tile([128, 128], BF16, tag="ident")
            nc.sync.dma_start(id_sb[:], ident[:])

            q_sb = qkvp.tile([128, HP, T], BF16, tag="q")
            k_sb = qkvp.tile([128, HP, T], BF16, tag="k")
            v_sb = qkvp.tile([128, HP, NTK, 130], BF16, tag="v")
            nc.vector.memset(v_sb[:, :, :, 64:65], 1.0)
            nc.vector.memset(v_sb[:, :, :, 129:130], 1.0)

            # ---- phase A: q^T, k^T, v for all head pairs ----
            with (
                tc.tile_pool(name="xtp", bufs=1) as xtp,
                tc.tile_pool(name="wst", bufs=3) as wst,
                tc.tile_pool(name="vtb", bufs=2) as vtp,
                tc.tile_pool(name="ps_a", bufs=6, space="PSUM") as ps_a,
                tc.tile_pool(name="ps_tr", bufs=2, space="PSUM") as ps_tr,
            ):
                w_tiles = []
                for hp in range(HP):
                    hs = slice(hp * 128, hp * 128 + 128)
                    wq_h = wst.tile([128, NCK, 128], BF16, tag=f"w{hp}", name=f"wq{hp}")
                    wk_h = wst.tile([128, NCK, 128], BF16, tag=f"w{hp}", name=f"wk{hp}")
                    wv_h = wst.tile([128, NCK, 128], BF16, tag=f"w{hp}", name=f"wv{hp}")
                    nc.sync.dma_start(wq_h[:], wq[:, :, hs].transpose([1, 0, 2]))
                    nc.sync.dma_start(wk_h[:], wk[:, :, hs].transpose([1, 0, 2]))
                    nc.sync.dma_start(wv_h[:], wv[:, :, hs].transpose([1, 0, 2]))
                    w_tiles.append((wq_h, wk_h, wv_h))
                    if hp == 0:
                        xt_sb = xtp.tile([128, NCK, T], BF16, tag="xt")
                        for win in range(4):
                            wsl = slice(T // 4 * win, T // 4 * (win + 1))
                            for ck in range(NCK):
                                nc.sync.dma_start(xt_sb[:, ck, wsl], xt[ck, :, wsl])
                for hp in range(HP):
                    wq_h, wk_h, wv_h = w_tiles[hp]
                    for i in range(NI):
                        ts = slice(512 * i, 512 * i + 512)
                        pq = ps_a.tile([128, 512], F32, tag="mm")
                        for ck in range(NCK):
                            nc.tensor.matmul(pq[:], wq_h[:, ck, :], xt_sb[:, ck, ts],
                                             start=(ck == 0), stop=(ck == NCK - 1))
                        nc.vector.tensor_copy(q_sb[:, hp, ts], pq[:])
                        pk = ps_a.tile([128, 512], F32, tag="mm")
                        for ck in range(NCK):
                            nc.tensor.matmul(pk[:], wk_h[:, ck, :], xt_sb[:, ck, ts],
                                             start=(ck == 0), stop=(ck == NCK - 1))
                        nc.vector.tensor_copy(k_sb[:, hp, ts], pk[:])
                        pv = ps_a.tile([128, 512], F32, tag="mm")
                        for ck in range(NCK):
                            nc.tensor.matmul(pv[:], wv_h[:, ck, :], xt_sb[:, ck, ts],
                                             start=(ck == 0), stop=(ck == NCK - 1))
                        vt_bf = vtp.tile([128, 512], BF16, tag="vt")
                        nc.vector.tensor_copy(vt_bf[:], pv[:])
                        for f in range(4):
                            pt = ps_tr.tile([128, 128], BF16, tag="tr")
                            nc.tensor.transpose(pt[:], vt_bf[:, 128 * f:128 * f + 128],
                                                id_sb[:])
                            nc.vector.tensor_copy(v_sb[:, hp, 4 * i + f, 0:64],
                                                  pt[:, 0:64])
                            nc.vector.tensor_copy(v_sb[:, hp, 4 * i + f, 65:129],
                                                  pt[:, 64:128])

            # ---- phase B: attention + projection ----
            with (
                tc.tile_pool(name="yp", bufs=1) as yp,
                tc.tile_pool(name="cst", bufs=1) as cst,
                tc.tile_pool(name="att", bufs=12) as attp,
                tc.tile_pool(name="nrm", bufs=2) as nrm,
                tc.tile_pool(name="outp", bufs=4) as outp,
                tc.tile_pool(name="ps_s", bufs=2, space="PSUM") as ps_s,
                tc.tile_pool(name="ps_y", bufs=2, space="PSUM") as ps_y,
            ):
                y_t = [yp.tile([128, T], BF16, tag=f"y{hp}", name=f"y{hp}")
                       for hp in range(HP)]
                wp_sb = cst.tile([128, HP, C], BF16, tag="wp")
                for hp in range(HP):
                    nc.sync.dma_start(wp_sb[:, hp, :], wp[hp])
                tri_sb = cst.tile([128, 256], BF16, tag="tri")
                nc.sync.dma_start(tri_sb[:], tri[:])

                def emit_proj(j, fs):
                    # projection for query blocks fs of block j (y_sb ready)
                    for f in fs:
                        t = 4 * j + f
                        ysl = slice(128 * t, 128 * t + 128)
                        ot = outp.tile([128, C], F32, tag="ot")
                        po = ps_s.tile([128, 1024], F32, tag="s", name=f"po{t}")
                        for hp2 in range(HP):
                            for ch in range(C // 512):
                                nc.tensor.matmul(po[:, 512 * ch:512 * ch + 512],
                                                 y_t[hp2][:, ysl],
                                                 wp_sb[:, hp2, 512 * ch:512 * ch + 512],
                                                 start=(hp2 == 0), stop=(hp2 == HP - 1),
                                                 skip_group_check=True)
                        nc.vector.tensor_copy(ot[:, 0:512], po[:, 0:512])
                        nc.vector.tensor_copy(ot[:, 512:1024], po[:, 512:1024])
                        nc.sync.dma_start(out[128 * t:128 * t + 128, :], ot[:])

                pending = None
                for j in range(NI):
                    ntk = 4 * j + 4
                    for hp in range(HP):
                        pyd = ps_y.tile([128, 1024], F32, tag="yd")
                        att_tiles = {}

                        def emit_attv(tkb, att_tiles=att_tiles, pyd=pyd,
                                      hp=hp, ntk=ntk, j=j):
                            r = tkb - 4 * j
                            co = 128 * r if r > 0 else 0
                            att = att_tiles.pop(tkb)
                            st = (tkb == 0)
                            sp = (tkb == ntk - 1)
                            nc.tensor.matmul(pyd[0:65, co:512],
                                             v_sb[:, hp, tkb, 0:65],
                                             att[:, 0, co:512], start=st, stop=sp,
                                             skip_group_check=True)
                            nc.tensor.matmul(pyd[0:65, 512 + co:1024],
                                             v_sb[:, hp, tkb, 65:130],
                                             att[:, 1, co:512], start=st, stop=sp,
                                             skip_group_check=True)

                        for tkb in range(ntk):
                            r = tkb - 4 * j
                            co = 128 * r if r > 0 else 0
                            ks = slice(128 * tkb, 128 * tkb + 128)
                            qs = slice(512 * j + co, 512 * j + 512)
                            pss = ps_s.tile([128, 1024], F32, tag="s")
                            nc.tensor.matmul(pss[:, co:512], k_sb[0:64, hp, ks],
                                             q_sb[0:64, hp, qs],
                                             start=True, stop=True, tile_position=(0, 0),
                                             skip_group_check=True)
                            nc.tensor.matmul(pss[:, 512 + co:1024], k_sb[64:128, hp, ks],
                                             q_sb[64:128, hp, qs],
                                             start=True, stop=True, tile_position=(64, 0),
                                             skip_group_check=True)
                            att = attp.tile([128, 2, 512], BF16, tag="att")
                            att_tiles[tkb] = att
                            pv2 = pss[:].rearrange("p (h t) -> p h t", h=2)
                            nc.scalar.activation(
                                att[:, :, co:512], pv2[:, :, co:512],
                                mybir.ActivationFunctionType.Exp, scale=0.125)
                            if r >= 0:
                                nc.vector.tensor_mul(
                                    att[:, :, co:co + 128],
                                    att[:, :, co:co + 128],
                                    tri_sb[:].rearrange("p (h t) -> p h t", h=2))
                            # defer previous block's projection into this
                            # block's score stream so Act never idles;
                            # batch att*v per 2 key blocks to halve PE
                            # mode switches (64-row scores <-> 128 att*v)
                            if tkb == 0 and pending is not None:
                                pending()
                                pending = None
                            if tkb % 2 == 1:
                                if tkb >= 3:
                                    emit_attv(tkb - 3)
                                    emit_attv(tkb - 2)
                                if j > 0 and hp == 1 and tkb == 1:
                                    emit_proj(j - 1, (0, 1))
                                if j > 0 and hp == 1 and tkb == 3:
                                    emit_proj(j - 1, (2, 3))

                        def emit_tail(emit_attv=emit_attv, pyd=pyd, hp=hp,
                                      ntk=ntk, j=j):
                            emit_attv(ntk - 2)
                            emit_attv(ntk - 1)

                            # ---- normalization ----
                            # yu rows 0:64 = unnormalized y, row 64 = dens
                            yu = nrm.tile([65, 1024], BF16, tag="yu")
                            nc.vector.tensor_copy(yu[:], pyd[0:65, :])
                            den8b = nrm.tile([128, 8], BF16, tag="den8b")
                            nc.sync.dma_start(den8b[:], yu[64:65, :])
                            den8 = nrm.tile([128, 8], F32, tag="den8")
                            nc.vector.tensor_copy(den8[:], den8b[:])
                            rec8 = nrm.tile([128, 8], F32, tag="rec8")
                            nc.vector.reciprocal(rec8[:], den8[:])
                            recrow = nrm.tile([1, 1024], F32, tag="recrow")
                            nc.sync.dma_start(recrow[:], rec8[:])
                            dT = nrm.tile([64, 1024], F32, tag="dT")
                            nc.gpsimd.partition_broadcast(dT[:], recrow[0:1, :])
                            recT = nrm.tile([128, 512], F32, tag="recT")
                            nc.sync.dma_start(recT[64:128, :], dT[0:64, 512:1024])
                            tqs = slice(512 * j, 512 * j + 512)
                            nc.vector.tensor_mul(y_t[hp][0:64, tqs],
                                                 yu[0:64, 0:512], dT[0:64, 0:512])
                            nc.sync.dma_start(y_t[hp][64:128, tqs],
                                              yu[0:64, 512:1024])
                            nc.vector.tensor_mul(y_t[hp][64:128, tqs],
                                                 y_t[hp][64:128, tqs],
                                                 recT[64:128, :])

                        pending = emit_tail

                pending()
                emit_proj(NI - 1, (0, 1, 2, 3))

    nc.compile()
    return nc


def make_inputs(x_b, w_qkv, w_proj, g, HL=8):
    """Host-side prep of one core's input map.

    x_b: [T, C] fp32 (one batch), g: head-group index (0 or 1).
    """
    import ml_dtypes
    BF = ml_dtypes.bfloat16
    T, C = x_b.shape
    D = 64
    NCK = C // 128
    HP = HL // 2
    h0 = g * HL * D
    xt = np.ascontiguousarray(x_b.T.reshape(NCK, 128, T)).astype(BF)
    wq = np.ascontiguousarray(
        w_qkv[:, h0:h0 + HL * D].reshape(NCK, 128, HL * D)).astype(BF)
    wk = np.ascontiguousarray(
        w_qkv[:, C + h0:C + h0 + HL * D].reshape(NCK, 128, HL * D)).astype(BF)
    wv = np.ascontiguousarray(
        w_qkv[:, 2 * C + h0:2 * C + h0 + HL * D].reshape(NCK, 128, HL * D)).astype(BF)
    wp = np.ascontiguousarray(
        w_proj[h0:h0 + HL * D, :].reshape(HP, 128, C)).astype(BF)
    t1 = np.triu(np.ones((128, 128), dtype=np.float32))
    tri = np.concatenate([t1, t1], axis=1).astype(BF)
    ident = np.eye(128, dtype=np.float32).astype(BF)
    return {"xt": xt, "wq": wq, "wk": wk, "wv": wv, "wp": wp,
            "tri": tri, "ident": ident}


_NC_CACHE = {}


def kernel(x, w_qkv, w_proj):
    import numpy as np
    from concourse.bass_utils import run_bass_kernel_spmd

    x = np.ascontiguousarray(np.asarray(x, dtype=np.float32))
    w_qkv = np.ascontiguousarray(np.asarray(w_qkv, dtype=np.float32))
    w_proj = np.ascontiguousarray(np.asarray(w_proj, dtype=np.float32))
    B, T, C = x.shape

    key = (T, C)
    if key not in _NC_CACHE:
        _NC_CACHE[key] = build(T=T, HL=8, C=C)
    nc = _NC_CACHE[key]

    in_maps = [make_inputs(x[c // 2], w_qkv, w_proj, c % 2, HL=8) for c in range(8)]
    res = run_bass_kernel_spmd(nc, in_maps, core_ids=list(range(8)), trace=False)

    out = np.zeros((B, T, C), dtype=np.float32)
    for c in range(8):
        out[c // 2] += res.results[c]["out"]
    return out



# revision 3
# speedup vs baseline: 1.1538x; 1.1538x over previous
"""Causal self-attention kernel v4 for 8 Trainium2 NeuronCores (Bass/Tile).

B=4, T=2048, C=1024, 16 heads. 8 cores = 4 batches x 2 head-groups (8 heads
each); host sums the two projection partials per batch.

Changes vs v3 (330913ns):
- v computed pre-transposed (lhsT = x^T chunks, rhs = w_v) -> kills the 64
  PE transposes + extra copies; v psum lands directly in [keys, d] layout
- single fused schedule: qkv tile matmuls for later head-pairs are emitted
  as "filler" inside the attention block loop, so the PE never idles at the
  phase boundary, never cold-throttles, and absorbs the Act-engine exp
  latency per block
- proj psum moved to the qkv [128,512] pool so scores double-buffering in
  ps_s is never blocked by projection
- y/den psum single-buffered (WAR on the yu copy is ~2 blocks early)
"""


import numpy as np
import concourse.bass as bass
import concourse.tile as tile
from concourse import mybir, bacc

F32 = mybir.dt.float32
BF16 = mybir.dt.bfloat16


def build(T=2048, HL=8, C=1024):
    D = 64
    HP = HL // 2               # head pairs per core
    NCK = C // 128             # contraction chunks for qkv
    NI = T // 512              # 512-wide query blocks
    NTK = T // 128             # 128-wide key blocks

    nc = bacc.Bacc("TRN2", debug=False, num_devices=8)

    xt = nc.dram_tensor("xt", [NCK, 128, T], BF16, kind="ExternalInput")
    wq = nc.dram_tensor("wq", [NCK, 128, HL * D], BF16, kind="ExternalInput")
    wk = nc.dram_tensor("wk", [NCK, 128, HL * D], BF16, kind="ExternalInput")
    wv = nc.dram_tensor("wv", [NCK, 128, HL * D], BF16, kind="ExternalInput")
    wp = nc.dram_tensor("wp", [HP, 128, C], BF16, kind="ExternalInput")
    tri = nc.dram_tensor("tri", [128, 256], BF16, kind="ExternalInput")
    out = nc.dram_tensor("out", [T, C], F32, kind="ExternalOutput")

    with tile.TileContext(nc) as tc:
        with (
            tc.tile_pool(name="persist", bufs=1) as pers,
            tc.tile_pool(name="wqk", bufs=1) as wqkp,
            tc.tile_pool(name="att", bufs=12) as attp,
            tc.tile_pool(name="nrm", bufs=2) as nrm,
            tc.tile_pool(name="outp", bufs=4) as outp,
            tc.tile_pool(name="ps_mm", bufs=2, space="PSUM") as ps_mm,
            tc.tile_pool(name="ps_s", bufs=2, space="PSUM") as ps_s,
            tc.tile_pool(name="ps_y", bufs=1, space="PSUM") as ps_y,
        ):
            # ---- persistent SBUF ----
            xt_sb = pers.tile([128, NCK, T], BF16, tag="xt")
            q_sb = pers.tile([128, HP, T], BF16, tag="q")
            k_sb = pers.tile([128, HP, T], BF16, tag="k")
            v_sb = pers.tile([128, HP, NTK, 130], BF16, tag="v")
            wv_sb = pers.tile([128, NCK, HL * D], BF16, tag="wv")
            wp_sb = pers.tile([128, HP, C], BF16, tag="wp")
            tri_sb = pers.tile([128, 256], BF16, tag="tri")
            y_t = [pers.tile([128, T], BF16, tag=f"y{hp}", name=f"y{hp}")
                   for hp in range(HP)]
            wqk_t = {}
            for hp in range(HP):
                wqk_t[(0, hp)] = wqkp.tile([128, NCK, 128], BF16,
                                           tag=f"wq{hp}", name=f"wq{hp}")
                wqk_t[(1, hp)] = wqkp.tile([128, NCK, 128], BF16,
                                           tag=f"wk{hp}", name=f"wk{hp}")

            nc.vector.memset(v_sb[:, :, :, 64:65], 1.0)
            nc.vector.memset(v_sb[:, :, :, 129:130], 1.0)

            # ---- DMA staging (order matters: queue drains in order) ----
            def dma_w(qk, hp):
                src = wq if qk == 0 else wk
                hs = slice(hp * 128, hp * 128 + 128)
                nc.sync.dma_start(wqk_t[(qk, hp)][:],
                                  src[:, :, hs].transpose([1, 0, 2]))

            def dma_x(win):
                wsl = slice(T // 4 * win, T // 4 * (win + 1))
                for ck in range(NCK):
                    nc.sync.dma_start(xt_sb[:, ck, wsl], xt[ck, :, wsl])

            dma_w(0, 0)
            dma_w(1, 0)
            dma_x(0)
            for ck in range(NCK):
                nc.sync.dma_start(wv_sb[:, ck, :], wv[ck])
            nc.sync.dma_start(tri_sb[:], tri[:])
            dma_x(1)
            dma_w(0, 1)
            dma_w(1, 1)
            dma_x(2)
            for hp in range(HP):
                nc.sync.dma_start(wp_sb[:, hp, :], wp[hp])
            dma_x(3)
            for hp in (2, 3):
                dma_w(0, hp)
                dma_w(1, hp)

            # ---- phase-A work units (emitted inline or as filler) ----
            # NOTE: each unit opens AND closes its psum tile atomically, so
            # arbitrary interleaving of units never splits an open
            # accumulation across other ps_mm.tile() rotations.
            def qk_tile(qk, hp, i):
                dst = q_sb if qk == 0 else k_sb
                ts = slice(512 * i, 512 * i + 512)
                w_h = wqk_t[(qk, hp)]
                p = ps_mm.tile([128, 512], F32, tag="mm", name=f"p{qk}{hp}{i}")
                for ck in range(NCK):
                    nc.tensor.matmul(p[:], w_h[:, ck, :], xt_sb[:, ck, ts],
                                     start=(ck == 0), stop=(ck == NCK - 1),
                                     skip_group_check=True)
                nc.vector.tensor_copy(dst[:, hp, ts], p[:])

            def v_tile(t):
                # token-tile t (128 keys), all head pairs at once
                ts = slice(128 * t, 128 * t + 128)
                p = ps_mm.tile([128, 512], F32, tag="mm", name=f"pv{t}")
                for ck in range(NCK):
                    nc.tensor.matmul(p[:], xt_sb[:, ck, ts], wv_sb[:, ck, :],
                                     start=(ck == 0), stop=(ck == NCK - 1),
                                     skip_group_check=True)
                pv = p[:].rearrange("p (h c) -> p h c", h=HP)
                nc.vector.tensor_copy(v_sb[:, :, t, 0:64], pv[:, :, 0:64])
                nc.vector.tensor_copy(v_sb[:, :, t, 65:129], pv[:, :, 64:128])

            # ---- prelude: enough phase A for (j0, hp0) ----
            for qk in (0, 1):
                qk_tile(qk, 0, 0)
            for t in range(4):
                v_tile(t)

            # ---- filler schedule: (j, hp) -> list of closures ----
            def QK(qk, hp, i):
                return [lambda: qk_tile(qk, hp, i)]

            def V(t):
                return [lambda: v_tile(t)]

            fill = {}
            fill[(0, 0)] = QK(0, 1, 0) + QK(1, 1, 0)
            fill[(0, 1)] = QK(0, 2, 0) + QK(1, 2, 0)
            fill[(0, 2)] = QK(0, 3, 0) + QK(1, 3, 0)
            fill[(0, 3)] = QK(0, 0, 1) + QK(1, 0, 1) + V(4) + V(5)
            fill[(1, 0)] = V(6) + V(7) + QK(0, 1, 1) + QK(1, 1, 1)
            fill[(1, 1)] = QK(0, 2, 1) + QK(1, 2, 1)
            fill[(1, 2)] = QK(0, 3, 1) + QK(1, 3, 1)
            fill[(1, 3)] = QK(0, 0, 2) + QK(1, 0, 2) + V(8) + V(9)
            fill[(2, 0)] = V(10) + V(11) + QK(0, 1, 2) + QK(1, 1, 2)
            fill[(2, 1)] = QK(0, 2, 2) + QK(1, 2, 2)
            fill[(2, 2)] = QK(0, 3, 2) + QK(1, 3, 2)
            fill[(2, 3)] = QK(0, 0, 3) + QK(1, 0, 3) + V(12) + V(13)
            fill[(3, 0)] = V(14) + V(15) + QK(0, 1, 3) + QK(1, 1, 3)
            fill[(3, 1)] = QK(0, 2, 3) + QK(1, 2, 3)
            fill[(3, 2)] = QK(0, 3, 3) + QK(1, 3, 3)
            fill[(3, 3)] = []

            def emit_proj(j, fs):
                # projection for query blocks fs of block j (y_sb ready)
                for f in fs:
                    t = 4 * j + f
                    ysl = slice(128 * t, 128 * t + 128)
                    ot = outp.tile([128, C], F32, tag="ot")
                    for ch in range(C // 512):
                        po = ps_mm.tile([128, 512], F32, tag="mm",
                                        name=f"po{t}_{ch}")
                        for hp2 in range(HP):
                            nc.tensor.matmul(po[:],
                                             y_t[hp2][:, ysl],
                                             wp_sb[:, hp2, 512 * ch:512 * ch + 512],
                                             start=(hp2 == 0), stop=(hp2 == HP - 1),
                                             skip_group_check=True)
                        nc.vector.tensor_copy(ot[:, 512 * ch:512 * ch + 512], po[:])
                    nc.sync.dma_start(out[128 * t:128 * t + 128, :], ot[:])

            # ---- attention + projection, with filler interleaved ----
            pending = None
            for j in range(NI):
                ntk = 4 * j + 4
                for hp in range(HP):
                    seg = list(fill[(j, hp)])
                    nseg = len(seg)
                    spread = max(1, (3 * ntk) // 4)

                    def pump(tkb, seg=seg, nseg=nseg, spread=spread):
                        want = (nseg * (tkb + 1) + spread - 1) // spread
                        while seg and (nseg - len(seg)) < min(want, nseg):
                            seg.pop(0)()

                    pyd = ps_y.tile([128, 1024], F32, tag="yd")
                    att_tiles = {}

                    def emit_attv(tkb, att_tiles=att_tiles, pyd=pyd,
                                  hp=hp, ntk=ntk, j=j):
                        r = tkb - 4 * j
                        co = 128 * r if r > 0 else 0
                        att = att_tiles.pop(tkb)
                        st = (tkb == 0)
                        sp = (tkb == ntk - 1)
                        nc.tensor.matmul(pyd[0:65, co:512],
                                         v_sb[:, hp, tkb, 0:65],
                                         att[:, 0, co:512], start=st, stop=sp,
                                         skip_group_check=True)
                        nc.tensor.matmul(pyd[0:65, 512 + co:1024],
                                         v_sb[:, hp, tkb, 65:130],
                                         att[:, 1, co:512], start=st, stop=sp,
                                         skip_group_check=True)

                    for tkb in range(ntk):
                        r = tkb - 4 * j
                        co = 128 * r if r > 0 else 0
                        ks = slice(128 * tkb, 128 * tkb + 128)
                        qs = slice(512 * j + co, 512 * j + 512)
                        pss = ps_s.tile([128, 1024], F32, tag="s")
                        nc.tensor.matmul(pss[:, co:512], k_sb[0:64, hp, ks],
                                         q_sb[0:64, hp, qs],
                                         start=True, stop=True, tile_position=(0, 0),
                                         skip_group_check=True)
                        nc.tensor.matmul(pss[:, 512 + co:1024], k_sb[64:128, hp, ks],
                                         q_sb[64:128, hp, qs],
                                         start=True, stop=True, tile_position=(64, 0),
                                         skip_group_check=True)
                        att = attp.tile([128, 2, 512], BF16, tag="att")
                        att_tiles[tkb] = att
                        pv2 = pss[:].rearrange("p (h t) -> p h t", h=2)
                        nc.scalar.activation(
                            att[:, :, co:512], pv2[:, :, co:512],
                            mybir.ActivationFunctionType.Exp, scale=0.125)
                        if r >= 0:
                            nc.vector.tensor_mul(
                                att[:, :, co:co + 128],
                                att[:, :, co:co + 128],
                                tri_sb[:].rearrange("p (h t) -> p h t", h=2))
                        # defer previous block's tail into this block's score
                        # stream so Act never idles; batch att*v per 2 key
                        # blocks to halve PE mode switches
                        if tkb == 0 and pending is not None:
                            pending()
                            pending = None
                        if tkb % 2 == 1:
                            if tkb >= 3:
                                emit_attv(tkb - 3)
                                emit_attv(tkb - 2)
                            if j > 0 and hp == 1 and tkb == 1:
                                emit_proj(j - 1, (0, 1))
                            if j > 0 and hp == 1 and tkb == 3:
                                emit_proj(j - 1, (2, 3))
                        pump(tkb)

                    def emit_tail(emit_attv=emit_attv, pyd=pyd, hp=hp,
                                  ntk=ntk, j=j):
                        emit_attv(ntk - 2)
                        emit_attv(ntk - 1)

                        # ---- normalization ----
                        # yu rows 0:64 = unnormalized y, row 64 = dens
                        yu = nrm.tile([65, 1024], BF16, tag="yu")
                        nc.vector.tensor_copy(yu[:], pyd[0:65, :])
                        den8b = nrm.tile([128, 8], BF16, tag="den8b")
                        nc.sync.dma_start(den8b[:], yu[64:65, :])
                        den8 = nrm.tile([128, 8], F32, tag="den8")
                        nc.vector.tensor_copy(den8[:], den8b[:])
                        rec8 = nrm.tile([128, 8], F32, tag="rec8")
                        nc.vector.reciprocal(rec8[:], den8[:])
                        recrow = nrm.tile([1, 1024], F32, tag="recrow")
                        nc.sync.dma_start(recrow[:], rec8[:])
                        dT = nrm.tile([64, 1024], F32, tag="dT")
                        nc.gpsimd.partition_broadcast(dT[:], recrow[0:1, :])
                        recT = nrm.tile([128, 512], F32, tag="recT")
                        nc.sync.dma_start(recT[64:128, :], dT[0:64, 512:1024])
                        tqs = slice(512 * j, 512 * j + 512)
                        nc.vector.tensor_mul(y_t[hp][0:64, tqs],
                                             yu[0:64, 0:512], dT[0:64, 0:512])
                        nc.sync.dma_start(y_t[hp][64:128, tqs],
                                          yu[0:64, 512:1024])
                        nc.vector.tensor_mul(y_t[hp][64:128, tqs],
                                             y_t[hp][64:128, tqs],
                                             recT[64:128, :])

                    pending = emit_tail

            pending()
            emit_proj(NI - 1, (0, 1, 2, 3))

    nc.compile()
    return nc


def make_inputs(x_b, w_qkv, w_proj, g, HL=8):
    """Host-side prep of one core's input map.

    x_b: [T, C] fp32 (one batch), g: head-group index (0 or 1).
    """
    import ml_dtypes
    BF = ml_dtypes.bfloat16
    T, C = x_b.shape
    D = 64
    NCK = C // 128
    HP = HL // 2
    h0 = g * HL * D
    xt = np.ascontiguousarray(x_b.T.reshape(NCK, 128, T)).astype(BF)
    wq = np.ascontiguousarray(
        w_qkv[:, h0:h0 + HL * D].reshape(NCK, 128, HL * D)).astype(BF)
    wk = np.ascontiguousarray(
        w_qkv[:, C + h0:C + h0 + HL * D].reshape(NCK, 128, HL * D)).astype(BF)
    wv = np.ascontiguousarray(
        w_qkv[:, 2 * C + h0:2 * C + h0 + HL * D].reshape(NCK, 128, HL * D)).astype(BF)
    wp = np.ascontiguousarray(
        w_proj[h0:h0 + HL * D, :].reshape(HP, 128, C)).astype(BF)
    t1 = np.triu(np.ones((128, 128), dtype=np.float32))
    tri = np.concatenate([t1, t1], axis=1).astype(BF)
    return {"xt": xt, "wq": wq, "wk": wk, "wv": wv, "wp": wp, "tri": tri}


_NC_CACHE = {}


def kernel(x, w_qkv, w_proj):
    import numpy as np
    from concourse.bass_utils import run_bass_kernel_spmd

    x = np.ascontiguousarray(np.asarray(x, dtype=np.float32))
    w_qkv = np.ascontiguousarray(np.asarray(w_qkv, dtype=np.float32))
    w_proj = np.ascontiguousarray(np.asarray(w_proj, dtype=np.float32))
    B, T, C = x.shape

    key = (T, C)
    if key not in _NC_CACHE:
        _NC_CACHE[key] = build(T=T, HL=8, C=C)
    nc = _NC_CACHE[key]

    in_maps = [make_inputs(x[c // 2], w_qkv, w_proj, c % 2, HL=8) for c in range(8)]
    res = run_bass_kernel_spmd(nc, in_maps, core_ids=list(range(8)), trace=False)

    out = np.zeros((B, T, C), dtype=np.float32)
    for c in range(8):
        out[c // 2] += res.results[c]["out"]
    return out
